# revision 1
# baseline (speedup 1.0000x reference)
"""CrossAttentionFusion Trainium2 kernel.

Full inputs -> shard (batch x query-half) over 8 NeuronCores -> full output.

Per core (batch b = core//2, query half h = core%2, NH=2048 queries):
  Algebraic folding (host precompute):
    L[m,n] = K^T Q = x2^T (k_w^T q_w) x1 =: x2^T Q'   (K never materialized;
             terms constant in m cancel in softmax; x2^T k_w^T q_b folds
             into Q' channel bias)
    F_att   = v_w (x2 A_norm) + v_b  ->  M1 = (proj_w v_w) Z,  Z = x2 E
             (V never materialized; proj_w v_w and proj_w v_b precomputed)
  Device per 512-query block:
    L[m, n] = x2^T Q'                (fp32r matmuls, m on partitions)
    E = exp(L / 16)                  (ACT; no max subtraction: logits O(1))
    S[n] = sum_m E[m, n]             (DVE running sum + one PE reduce)
    Z[c, n] = sum_m x2[c, m] E[m, n] (lhsT = host-pretransposed x2)
    M1 = P2 Z ;  out = x1 + gate * relu(M1 * G * (1/S) + Bc)
  with G = gamma*rsqrt(var+eps), Bc = beta + (proj_b + proj_w v_b - mean)*G.
  fusion(j-1) is interleaved into logits(j) on the PE; exp and the softmax
  sum run on ACT/DVE one step behind; 1/S is hidden under the next block.

Everything on the PE runs in float32r (~2e-4 matmul rel err, full rate).
"""
from contextlib import ExitStack

import numpy as np

import concourse.bass as bass
import concourse.mybir as mybir
import concourse.tile as tile
from concourse import bacc
from concourse.bass_utils import run_bass_kernel_spmd

F32 = mybir.dt.float32
F32R = mybir.dt.float32r
AF = mybir.ActivationFunctionType
OP = mybir.AluOpType

B, C, H, W = 4, 256, 64, 64
N = H * W            # 4096
NCORES = 8
NH = N // 2          # 2048 queries per core
NBLK = 512           # query block
NBLOCKS = NH // NBLK
MT = N // 128        # 32 m-tiles
EPS = 1e-5
SCALE = float(C) ** -0.5


def build():
    nc = bacc.Bacc("TRN2", target_bir_lowering=False, debug=False,
                   num_devices=NCORES)
    x1r_d = nc.dram_tensor("x1r", [C, NH], F32R, kind="ExternalInput")
    x2r_d = nc.dram_tensor("x2r", [C, N], F32R, kind="ExternalInput")
    x2t_d = nc.dram_tensor("x2t", [128, MT * C], F32R, kind="ExternalInput")
    wm_d = nc.dram_tensor("wmat", [C, 2 * C], F32R, kind="ExternalInput")
    gw_d = nc.dram_tensor("gw", [C, 2], F32R, kind="ExternalInput")
    vec_d = nc.dram_tensor("vecs", [C, 4], F32, kind="ExternalInput")
    gb_d = nc.dram_tensor("gateb", [1, 1], F32, kind="ExternalInput")
    out_d = nc.dram_tensor("out", [C, NH], F32, kind="ExternalOutput")

    with tile.TileContext(nc) as tc, ExitStack() as ctx:
        pers = ctx.enter_context(tc.tile_pool(name="pers", bufs=1))
        work = ctx.enter_context(tc.tile_pool(name="work", bufs=2))
        psum = ctx.enter_context(tc.tile_pool(name="psum", bufs=1, space="PSUM"))

        # ---- persistent tiles ----
        wm = [pers.tile([128, 2 * C], F32R, tag=f"wm{ci}", name=f"wm{ci}") for ci in range(2)]
        gw = [pers.tile([128, 2], F32R, tag=f"gw{ci}", name=f"gw{ci}") for ci in range(2)]
        vec = [pers.tile([128, 4], F32, tag=f"vec{ci}", name=f"vec{ci}") for ci in range(2)]
        gb = pers.tile([1, 1], F32, tag="gb", name="gb")
        x2r = [pers.tile([128, N], F32R, tag=f"x2r{ci}", name=f"x2r{ci}") for ci in range(2)]
        x2t = pers.tile([128, MT * C], F32R, tag="x2t", name="x2t")
        Qt = [pers.tile([128, NH], F32R, tag=f"Qt{co}", name=f"Qt{co}") for co in range(2)]
        grow = pers.tile([1, NH], F32R, tag="grow", name="grow")
        ones_f = pers.tile([128, 1], F32, tag="ones_f", name="ones_f")
        ones_f2 = pers.tile([1, 128], F32, tag="ones_f2", name="ones_f2")
        ones_c = pers.tile([128, 1], F32R, tag="ones_c", name="ones_c")
        ones_k1 = pers.tile([1, 128], F32R, tag="ones_k1", name="ones_k1")

        # E pool created before xin so both coexist (budgeted); xin's
        # release after gate frees its space for good.
        epool = ctx.enter_context(tc.tile_pool(name="epool", bufs=1))
        E = epool.tile([128, MT * NBLK], F32R, tag="E", name="E")

        def fusion_mms(fp, mt):
            es = slice(mt * NBLK, (mt + 1) * NBLK)
            for co in range(2):
                nc.tensor.matmul(
                    fp[co][:], x2t[:, mt * C + co * 128: mt * C + (co + 1) * 128],
                    E[:, es], start=(mt == 0), stop=(mt == MT - 1))

        def sacc_adds(sacc, mt2):
            e0 = slice((2 * mt2) * NBLK, (2 * mt2 + 1) * NBLK)
            e1 = slice((2 * mt2 + 1) * NBLK, (2 * mt2 + 2) * NBLK)
            if mt2 == 0:
                nc.vector.tensor_add(sacc[:], E[:, e0], E[:, e1])
            else:
                nc.vector.tensor_add(sacc[:], sacc[:], E[:, e0])
                nc.vector.tensor_add(sacc[:], sacc[:], E[:, e1])

        def s_finalize(j, sacc):
            with nc.named_scope(f"sfin{j}"):
                sp = psum.tile([1, NBLK], F32, tag="s", name="s", bufs=1)
                nc.tensor.matmul(sp[:], ones_c[:], sacc[:])
                invs_f = work.tile([1, NBLK], F32, tag="invs_f", name="invs_f",
                                   bufs=1)
                nc.vector.reciprocal_approx_fast(invs_f[:], sp[:])
                invs_r = work.tile([1, NBLK], F32R, tag="invs_r", name="invs_r",
                                   bufs=1)
                nc.vector.tensor_copy(invs_r[:], invs_f[:])
            return invs_r

        def post_block(j, fp, invs_r):
            ns = slice(j * NBLK, (j + 1) * NBLK)
            with nc.named_scope(f"post{j}"):
                Fs = [work.tile([128, NBLK], F32R, tag=f"Fs{co}", name=f"Fs{co}",
                                bufs=1) for co in range(2)]
                for co in range(2):
                    nc.scalar.activation(Fs[co][:], fp[co][:], AF.Copy)
                bc1 = psum.tile([128, NBLK], F32, tag="acc", name="acc", bufs=3)
                nc.tensor.matmul(bc1[:], ones_k1[:], invs_r[:])
                invs_b = work.tile([128, NBLK], F32, tag="invs_b", name="invs_b",
                                   bufs=1)
                nc.vector.tensor_copy(invs_b[:], bc1[:])
                bc2 = psum.tile([128, NBLK], F32, tag="acc", name="acc", bufs=3)
                nc.tensor.matmul(bc2[:], ones_k1[:], grow[:, ns])
                gate_b = work.tile([128, NBLK], F32, tag="gate_b", name="gate_b",
                                   bufs=1)
                nc.vector.tensor_copy(gate_b[:], bc2[:])
                for co in range(2):
                    cs = slice(co * 128, (co + 1) * 128)
                    mp = psum.tile([128, NBLK], F32, tag="acc", name="acc", bufs=3)
                    for ci in range(2):
                        nc.tensor.matmul(
                            mp[:], wm[ci][:, C + co * 128: C + (co + 1) * 128],
                            Fs[ci][:], start=(ci == 0), stop=(ci == 1))
                    x1t = work.tile([128, NBLK], F32R, tag="x1t", name="x1t")
                    nc.sync.dma_start(x1t[:], x1r_d[cs, ns])
                    t1 = work.tile([128, NBLK], F32, tag="t1", name="t1")
                    nc.vector.scalar_tensor_tensor(
                        t1[:], mp[:], vec[co][:, 1:2], invs_b[:],
                        op0=OP.mult, op1=OP.mult)
                    r = work.tile([128, NBLK], F32, tag="r", name="r")
                    nc.scalar.activation(r[:], t1[:], AF.Relu,
                                         bias=vec[co][:, 2:3])
                    rg = work.tile([128, NBLK], F32, tag="t1", name="rg")
                    nc.gpsimd.tensor_mul(rg[:], r[:], gate_b[:])
                    ot = work.tile([128, NBLK], F32, tag="ot", name="ot")
                    nc.gpsimd.tensor_add(ot[:], rg[:], x1t[:].bitcast(F32))
                    nc.sync.dma_start(out_d[cs, ns], ot[:])

        def emit_block(blk, prev_fp, sacc):
            ns = slice(blk * NBLK, (blk + 1) * NBLK)
            for mt2 in range(MT // 2):
                lp = psum.tile([128, 2 * NBLK], F32, tag="L", name="L", bufs=2)
                for sub in range(2):
                    mt = 2 * mt2 + sub
                    msl = slice(mt * 128, (mt + 1) * 128)
                    for ci in range(2):
                        nc.tensor.matmul(
                            lp[:, sub * NBLK:(sub + 1) * NBLK],
                            x2r[ci][:, msl], Qt[ci][:, ns],
                            start=(ci == 0), stop=(ci == 1))
                if prev_fp is not None:
                    fusion_mms(prev_fp, 2 * mt2)
                    fusion_mms(prev_fp, 2 * mt2 + 1)
                nc.scalar.activation(
                    E[:, mt2 * 2 * NBLK:(mt2 + 1) * 2 * NBLK], lp[:],
                    AF.Exp, scale=SCALE)
                if mt2 > 0:
                    sacc_adds(sacc, mt2 - 1)
            sacc_adds(sacc, MT // 2 - 1)

        with nc.named_scope("pre"):
            nc.sync.dma_start(wm[0][:], wm_d[0:128, :])
            nc.gpsimd.dma_start(wm[1][:], wm_d[128:256, :])
            nc.vector.memset(ones_f[:], 1.0)
            nc.vector.tensor_copy(ones_c[:], ones_f[:])
            nc.vector.memset(ones_f2[:], 1.0)
            nc.vector.tensor_copy(ones_k1[:], ones_f2[:])

        sacc0 = None
        with tc.tile_pool(name="xin", bufs=1) as xin:
            x1r = [xin.tile([128, NH], F32R, tag=f"x1r{ci}", name=f"x1r{ci}") for ci in range(2)]
            with nc.named_scope("pre"):
                CH = 1024
                # interleave x1/x2 chunks: Q' and logits0 stream against arrivals
                nc.sync.dma_start(x1r[0][:, 0:CH], x1r_d[0:128, 0:CH])
                nc.gpsimd.dma_start(x1r[1][:, 0:CH], x1r_d[128:256, 0:CH])
                nc.sync.dma_start(x2r[0][:, 0:CH], x2r_d[0:128, 0:CH])
                nc.gpsimd.dma_start(x2r[1][:, 0:CH], x2r_d[128:256, 0:CH])
                nc.sync.dma_start(x1r[0][:, CH:NH], x1r_d[0:128, CH:NH])
                nc.gpsimd.dma_start(x1r[1][:, CH:NH], x1r_d[128:256, CH:NH])
                for ch in range(1, N // CH):
                    chs = slice(ch * CH, (ch + 1) * CH)
                    nc.sync.dma_start(x2r[0][:, chs], x2r_d[0:128, chs])
                    nc.gpsimd.dma_start(x2r[1][:, chs], x2r_d[128:256, chs])
                for ci in range(2):
                    cs = slice(ci * 128, (ci + 1) * 128)
                    nc.sync.dma_start(gw[ci][:], gw_d[cs, :])
                    nc.sync.dma_start(vec[ci][:], vec_d[cs, :])
                nc.sync.dma_start(gb[:], gb_d[:])
                nc.sync.dma_start(x2t[:, 0: MT * C // 2], x2t_d[:, 0: MT * C // 2])
                nc.gpsimd.dma_start(x2t[:, MT * C // 2:], x2t_d[:, MT * C // 2:])

                # Q' projection
                for co in range(2):
                    for nch in range(NH // NBLK):
                        ns = slice(nch * NBLK, (nch + 1) * NBLK)
                        qp = psum.tile([128, NBLK], F32, tag="acc", name="acc", bufs=3)
                        for ci in range(2):
                            nc.tensor.matmul(
                                qp[:], wm[ci][:, co * 128:(co + 1) * 128],
                                x1r[ci][:, ns], start=(ci == 0), stop=(ci == 1))
                        nc.scalar.activation(Qt[co][:, ns], qp[:], AF.Identity,
                                             bias=vec[co][:, 0:1])
            with nc.named_scope("blk0"):
                sacc0 = work.tile([128, NBLK], F32R, tag="sacc", name="sacc",
                                  bufs=2)
                emit_block(0, None, sacc0)
            with nc.named_scope("gate"):
                # gate row (x2 columns pre-permuted: query pixels = 0..NH)
                for blk in range(NBLOCKS):
                    ns = slice(blk * NBLK, (blk + 1) * NBLK)
                    gp = psum.tile([1, NBLK], F32, tag="L", name="gp", bufs=2)
                    for ci in range(2):
                        nc.tensor.matmul(gp[:], gw[ci][:, 0:1], x1r[ci][:, ns],
                                         start=(ci == 0), stop=False)
                    for ci in range(2):
                        nc.tensor.matmul(gp[:], gw[ci][:, 1:2], x2r[ci][:, ns],
                                         start=False, stop=(ci == 1))
                    nc.scalar.activation(grow[:, ns], gp[:], AF.Sigmoid,
                                         bias=gb[:])

        prev_fp = None
        prev_sacc = sacc0
        prev_invs = None
        prev = 0
        for blk in range(1, NBLOCKS):
            with nc.named_scope(f"blk{blk}"):
                prev_invs = s_finalize(prev, prev_sacc)
                prev_fp = [psum.tile([128, NBLK], F32, tag="acc", name="acc",
                                     bufs=3) for _ in range(2)]
                sacc = work.tile([128, NBLK], F32R, tag="sacc", name="sacc",
                                 bufs=2)
                emit_block(blk, prev_fp, sacc)
            post_block(prev, prev_fp, prev_invs)
            prev = blk
            prev_sacc = sacc
        with nc.named_scope("tail"):
            prev_invs = s_finalize(prev, prev_sacc)
            prev_fp = [psum.tile([128, NBLK], F32, tag="acc", name="acc", bufs=3)
                       for _ in range(2)]
            for mt in range(MT):
                fusion_mms(prev_fp, mt)
        post_block(prev, prev_fp, prev_invs)
    nc.compile()
    return nc


_NC = None


def _get_nc():
    global _NC
    if _NC is None:
        _NC = build()
    return _NC


def kernel(**inputs):
    x1 = np.ascontiguousarray(np.asarray(inputs["x1"], dtype=np.float32)).reshape(B, C, N)
    x2 = np.ascontiguousarray(np.asarray(inputs["x2"], dtype=np.float32)).reshape(B, C, N)
    q_w = np.asarray(inputs["q_w"], np.float64)
    k_w = np.asarray(inputs["k_w"], np.float64)
    v_w = np.asarray(inputs["v_w"], np.float64)
    p_w = np.asarray(inputs["proj_w"], np.float64)
    q_b = np.asarray(inputs["q_b"], np.float64)
    v_b = np.asarray(inputs["v_b"], np.float64)
    p_b = np.asarray(inputs["proj_b"], np.float64)
    gamma = np.asarray(inputs["bn_gamma"], np.float64)
    beta = np.asarray(inputs["bn_beta"], np.float64)
    mean = np.asarray(inputs["bn_mean"], np.float64)
    var = np.asarray(inputs["bn_var"], np.float64)
    gate_w = np.asarray(inputs["gate_w"], np.float32)
    gate_b = np.asarray(inputs["gate_b"], np.float32)

    # folded weights: Q' = (k_w^T q_w) x1 + k_w^T q_b ;  M1 = (proj_w v_w) Z
    wqkT = (q_w.T @ k_w).astype(np.float32)          # lhsT for Q' projection
    p2T = (v_w.T @ p_w.T).astype(np.float32)         # lhsT for proj stage
    wmat = np.ascontiguousarray(np.concatenate([wqkT, p2T], axis=1))
    gw = np.ascontiguousarray(
        np.stack([gate_w[0, :C], gate_w[0, C:]], axis=1).astype(np.float32))
    G = gamma / np.sqrt(var + EPS)
    Bc = beta + (p_b + p_w @ v_b - mean) * G
    qpb = k_w.T @ q_b
    vecs = np.ascontiguousarray(
        np.stack([qpb, G, Bc, np.zeros(C)], axis=1).astype(np.float32))
    gb = gate_b.reshape(1, 1)

    in_maps = []
    for core in range(NCORES):
        b, half = divmod(core, 2)
        hq = slice(half * NH, (half + 1) * NH)
        ho = slice((1 - half) * NH, (2 - half) * NH)
        x1q = np.ascontiguousarray(x1[b][:, hq])
        x2p = np.ascontiguousarray(np.concatenate([x2[b][:, hq], x2[b][:, ho]],
                                                  axis=1))
        # x2 pretransposed into the fusion lhsT SBUF layout:
        # x2t[p, mt*C + c] = x2p[c, mt*128 + p]
        x2t = np.ascontiguousarray(
            x2p.reshape(C, MT, 128).transpose(2, 1, 0).reshape(128, MT * C))
        in_maps.append({
            "x1r": x1q, "x2r": x2p, "x2t": x2t,
            "wmat": wmat, "gw": gw, "vecs": vecs, "gateb": gb,
        })

    nc = _get_nc()
    res = run_bass_kernel_spmd(nc, in_maps, core_ids=list(range(NCORES)))
    out = np.empty((B, C, N), np.float32)
    for core in range(NCORES):
        b, half = divmod(core, 2)
        out[b, :, half * NH:(half + 1) * NH] = res.results[core]["out"]
    return out.reshape(B, C, H, W)



# revision 5
# speedup vs baseline: 1.1765x; 1.1765x over previous
"""CrossAttentionFusion Trainium2 kernel (v2).

Full inputs -> shard (batch x query-half) over 8 NeuronCores -> full output.

Per core (batch b = core//2, query half h = core%2, NH=2048 queries):
  Algebraic folding (host precompute):
    L[m,n] = K^T Q = x2^T (k_w^T q_w) x1 =: x2^T Q'   (K never materialized;
             x2^T k_w^T q_b folds into Q' channel bias)
    F_att   = v_w (x2 A_norm) + v_b  ->  M1 = (proj_w v_w) Z,  Z = x2 E
    gate    = sigmoid(z) = (1 + tanh(z/2)) / 2; the 1/2 folds into the BN
             constants so ACT never leaves the exp/tanh function table.
  Device per 512-query block j (fusion interleaved INTO the same block,
  trailing exp by 2 key-tile-pairs; E is a 4-slot ring, not a full buffer):
    L[m, ns] = x2^T Q'            (bf16 matmuls, keys m on partitions)
    E = exp(L / 16)               (ACT -> bf16; logits O(1), no max needed)
    Z[c, ns] = sum_m x2t[m,c] E[m, ns]   (bf16, accumulated over 32 m-tiles)
    S[ns] = sum_m E[m, ns]        (bf16 pairwise tree on DVE (2x mode), then
                                   ones[128,128] matmul -> S broadcast to all
                                   partitions; reciprocal on DVE)
    M1 = P2 Z ; r' = relu(M1*(G/2)*(1/S) + Bc/2)  (DVE STT + ACT relu-bias)
    out = x1 + (1+tanh((gz+gb)/2)) * r'           (Pool STT + adds)
  with G = gamma*rsqrt(var+eps), Bc = beta + (proj_b + proj_w v_b - mean)*G.
  x1 (fp32) stays resident for the residual; x2 ships as bf16 twice
  (channels-major for logits, keys-major pretransposed for fusion).
  DMA uses 3 DGE rings (SP + ACT + Pool) with chunks ordered by first use.
"""
from contextlib import ExitStack

import numpy as np
import ml_dtypes

import concourse.bass as bass
import concourse.mybir as mybir
import concourse.tile as tile
from concourse import bacc
from concourse.bass_utils import run_bass_kernel_spmd

F32 = mybir.dt.float32
F32R = mybir.dt.float32r
BF16 = mybir.dt.bfloat16
AF = mybir.ActivationFunctionType
OP = mybir.AluOpType

B, C, H, W = 4, 256, 64, 64
N = H * W            # 4096
NCORES = 8
NH = N // 2          # 2048 queries per core
NBLK = 512           # query block
NBLOCKS = NH // NBLK
MT = N // 128        # 32 key tiles
MT2 = MT // 2        # 16 exp steps per block
EPS = 1e-5
SCALE = float(C) ** -0.5


def build():
    nc = bacc.Bacc("TRN2", target_bir_lowering=False, debug=False,
                   num_devices=NCORES)
    x1r_d = nc.dram_tensor("x1r", [C, NH], F32R, kind="ExternalInput")
    x2rb_d = nc.dram_tensor("x2rb", [C, N], BF16, kind="ExternalInput")
    x2tb_d = nc.dram_tensor("x2tb", [128, MT * C], BF16, kind="ExternalInput")
    wq_d = nc.dram_tensor("wq", [C, C], F32R, kind="ExternalInput")
    p2_d = nc.dram_tensor("p2", [C, C], BF16, kind="ExternalInput")
    gwf_d = nc.dram_tensor("gwf", [C, 128], F32R, kind="ExternalInput")
    gwb_d = nc.dram_tensor("gwb", [C, 128], BF16, kind="ExternalInput")
    vec_d = nc.dram_tensor("vecs", [C, 4], F32, kind="ExternalInput")
    out_d = nc.dram_tensor("out", [C, NH], F32, kind="ExternalOutput")

    with tile.TileContext(nc) as tc, ExitStack() as ctx:
        pers = ctx.enter_context(tc.tile_pool(name="pers", bufs=1))
        work = ctx.enter_context(tc.tile_pool(name="work", bufs=2))
        psum = ctx.enter_context(tc.tile_pool(name="psum", bufs=1, space="PSUM"))

        # ---- persistent tiles ----
        wq = [pers.tile([128, C], F32R, tag=f"wq{ci}", name=f"wq{ci}") for ci in range(2)]
        p2 = [pers.tile([128, C], BF16, tag=f"p2{ci}", name=f"p2{ci}") for ci in range(2)]
        gwf = [pers.tile([128, 128], F32R, tag=f"gwf{ci}", name=f"gwf{ci}") for ci in range(2)]
        gwb = [pers.tile([128, 128], BF16, tag=f"gwb{ci}", name=f"gwb{ci}") for ci in range(2)]
        vec = [pers.tile([128, 4], F32, tag=f"vec{ci}", name=f"vec{ci}") for ci in range(2)]
        x1r = [pers.tile([128, NH], F32R, tag=f"x1r{ci}", name=f"x1r{ci}") for ci in range(2)]
        x2rb = [pers.tile([128, N], BF16, tag=f"x2rb{ci}", name=f"x2rb{ci}") for ci in range(2)]
        x2tb = pers.tile([128, MT * C], BF16, tag="x2tb", name="x2tb")
        Qt = [pers.tile([128, NH], BF16, tag=f"Qt{co}", name=f"Qt{co}") for co in range(2)]
        ones_f = pers.tile([128, 128], F32, tag="ones_f", name="ones_f")
        ones_b = pers.tile([128, 128], BF16, tag="ones_b", name="ones_b")

        XCH = 512
        NX1 = NH // XCH   # 4 chunks per ci
        NX2 = N // XCH    # 8 chunks per ci

        # ---------- pre: constants + input streaming (3 DGE rings) ----------
        # issue instructions cost ~0.6us on the issuing engine's queue, so
        # the ACT ring carries only fusion-side (x2tb/p2) loads.
        with nc.named_scope("pre"):
            nc.vector.memset(ones_f[:], 1.0)
            nc.vector.tensor_copy(ones_b[:], ones_f[:])
            c0, c1 = slice(0, 128), slice(128, 256)
            s0 = slice(0, XCH)
            # earliest needs: wq, x1 ch0 (-> Q'0), vec, x2 ch0, x2t ch0
            nc.sync.dma_start(wq[0][:], wq_d[c0, :])
            nc.gpsimd.dma_start(wq[1][:], wq_d[c1, :])
            nc.scalar.dma_start(x2tb[:, 0:4 * C], x2tb_d[:, 0:4 * C])
            nc.sync.dma_start(x1r[0][:, s0], x1r_d[c0, s0])
            nc.gpsimd.dma_start(x1r[1][:, s0], x1r_d[c1, s0])
            nc.sync.dma_start(vec[0][:], vec_d[c0, :])
            nc.gpsimd.dma_start(vec[1][:], vec_d[c1, :])
            nc.sync.dma_start(x2rb[0][:, s0], x2rb_d[c0, s0])
            nc.gpsimd.dma_start(x2rb[1][:, s0], x2rb_d[c1, s0])
            nc.sync.dma_start(gwf[0][:], gwf_d[c0, :])
            nc.gpsimd.dma_start(gwf[1][:], gwf_d[c1, :])
            nc.sync.dma_start(gwb[0][:], gwb_d[c0, :])
            nc.gpsimd.dma_start(gwb[1][:], gwb_d[c1, :])
            # stream x2 chunks (logits+fusion sides), x1 chunks woven in
            for ch in range(1, NX2):
                chs = slice(ch * XCH, (ch + 1) * XCH)
                nc.sync.dma_start(x2rb[0][:, chs], x2rb_d[c0, chs])
                nc.gpsimd.dma_start(x2rb[1][:, chs], x2rb_d[c1, chs])
                ts = slice(ch * 4 * C, (ch + 1) * 4 * C)
                nc.scalar.dma_start(x2tb[:, ts], x2tb_d[:, ts])
                if ch < NX1:
                    x1s = slice(ch * XCH, (ch + 1) * XCH)
                    nc.sync.dma_start(x1r[0][:, x1s], x1r_d[c0, x1s])
                    nc.gpsimd.dma_start(x1r[1][:, x1s], x1r_d[c1, x1s])
            nc.scalar.dma_start(p2[0][:], p2_d[c0, :])
            nc.scalar.dma_start(p2[1][:], p2_d[c1, :])

        def emit_qproj(nch):
            ns = slice(nch * NBLK, (nch + 1) * NBLK)
            for co in range(2):
                qp = psum.tile([128, NBLK], F32, tag="acc", name="acc", bufs=2)
                for ci in range(2):
                    nc.tensor.matmul(
                        qp[:], wq[ci][:, co * 128:(co + 1) * 128],
                        x1r[ci][:, ns], start=(ci == 0), stop=(ci == 1))
                nc.scalar.activation(Qt[co][:, ns], qp[:], AF.Identity,
                                     bias=vec[co][:, 0:1])

        def emit_gate(j):
            """Gate logits for block j, partition-broadcast via replicated
            gate-weight lhsT; tanh((z+gb)/2) -> tg [128,NBLK] fp32."""
            ns = slice(j * NBLK, (j + 1) * NBLK)
            gp = psum.tile([128, NBLK], F32, tag="acc", name="gp", bufs=2)
            for ci in range(2):
                nc.tensor.matmul(gp[:], gwf[ci][:], x1r[ci][:, ns],
                                 start=(ci == 0), stop=False)
            for ci in range(2):
                nc.tensor.matmul(gp[:], gwb[ci][:], x2rb[ci][:, ns],
                                 start=False, stop=(ci == 1))
            tg = work.tile([128, NBLK], F32, tag="tg", name="tg", bufs=2)
            nc.scalar.activation(tg[:], gp[:], AF.Tanh, scale=0.5,
                                 bias=vec[0][:, 3:4])
            return tg

        def s_finalize(j, sacc):
            """S (sum over keys) broadcast to all partitions, then 1/S."""
            with nc.named_scope(f"sfin{j}"):
                sb = psum.tile([128, NBLK], F32, tag="acc", name="sb", bufs=2)
                nc.tensor.matmul(sb[:], ones_b[:], sacc[:])
                invs = work.tile([128, NBLK], F32, tag="invs", name="invs",
                                 bufs=2)
                nc.vector.reciprocal_approx_fast(invs[:], sb[:])
            return invs

        def emit_m1(Fs, co):
            mp = psum.tile([128, NBLK], F32, tag="acc", name="acc", bufs=2)
            for ci in range(2):
                nc.tensor.matmul(mp[:], p2[ci][:, co * 128:(co + 1) * 128],
                                 Fs[ci][:], start=(ci == 0), stop=(ci == 1))
            return mp

        def post_co(j, co, mp, invs, tg):
            """Normalize + BN + relu + gate + residual + store for one co."""
            ns = slice(j * NBLK, (j + 1) * NBLK)
            cs = slice(co * 128, (co + 1) * 128)
            with nc.named_scope(f"post{j}_{co}"):
                t1 = work.tile([128, NBLK], F32, tag=f"t1{co}", name="t1")
                nc.vector.scalar_tensor_tensor(
                    t1[:], mp[:], vec[co][:, 1:2], invs[:],
                    op0=OP.mult, op1=OP.mult)
                r = work.tile([128, NBLK], F32, tag=f"r{co}", name="r")
                nc.scalar.activation(r[:], t1[:], AF.Relu,
                                     bias=vec[co][:, 2:3])
                rg = work.tile([128, NBLK], F32, tag=f"rg{co}", name="rg")
                nc.vector.scalar_tensor_tensor(rg[:], tg[:], 1.0, r[:],
                                               op0=OP.add, op1=OP.mult)
                ot = work.tile([128, NBLK], F32, tag=f"ot{co}", name="ot")
                nc.gpsimd.tensor_add(ot[:], rg[:],
                                     x1r[co][:, ns].bitcast(F32))
                nc.sync.dma_start(out_d[cs, ns], ot[:])

        def emit_block(j, boundary):
            """Logits+exp+fusion for block j; fusion trails exp by 2 steps.
            boundary(k) emits interleaved PE work after logits step k."""
            ns = slice(j * NBLK, (j + 1) * NBLK)
            slots = [None] * 8

            def feed(t, lvl):
                if slots[lvl] is None:
                    slots[lvl] = t
                    return
                prev = slots[lvl]
                slots[lvl] = None
                nt = work.tile([128, NBLK], BF16, tag=f"tree{lvl}",
                               name=f"tree{lvl}", bufs=2)
                nc.vector.tensor_add(nt[:], prev[:], t[:])
                feed(nt, lvl + 1)

            fp = [psum.tile([128, NBLK], F32, tag=f"F{co}", name=f"F{co}",
                            bufs=1) for co in range(2)]
            Ets = [None] * MT2

            def fusion_step(mt2):
                Et = Ets[mt2]
                for sub in range(2):
                    mt = 2 * mt2 + sub
                    es = slice(sub * NBLK, (sub + 1) * NBLK)
                    for co in range(2):
                        nc.tensor.matmul(
                            fp[co][:],
                            x2tb[:, mt * C + co * 128: mt * C + (co + 1) * 128],
                            Et[:, es], start=(mt == 0), stop=(mt == MT - 1))

            for mt2 in range(MT2):
                lp = psum.tile([128, 2 * NBLK], F32, tag="L", name="L", bufs=2)
                for sub in range(2):
                    mt = 2 * mt2 + sub
                    msl = slice(mt * 128, (mt + 1) * 128)
                    for ci in range(2):
                        nc.tensor.matmul(
                            lp[:, sub * NBLK:(sub + 1) * NBLK],
                            x2rb[ci][:, msl], Qt[ci][:, ns],
                            start=(ci == 0), stop=(ci == 1))
                if boundary is not None:
                    boundary(mt2)
                Et = work.tile([128, 2 * NBLK], BF16, tag="E", name="E",
                               bufs=4)
                nc.scalar.activation(Et[:], lp[:], AF.Exp, scale=SCALE)
                Ets[mt2] = Et
                if mt2 >= 2:
                    fusion_step(mt2 - 2)
                # softmax-sum tree (DVE, bf16 2x): pair within Et, then fold
                p1 = work.tile([128, NBLK], BF16, tag="tree1", name="tree1",
                               bufs=2)
                nc.vector.tensor_add(p1[:], Et[:, 0:NBLK], Et[:, NBLK:2 * NBLK])
                feed(p1, 2)
            fusion_step(MT2 - 2)
            fusion_step(MT2 - 1)
            sacc = slots[6]
            assert sacc is not None
            return fp, sacc

        tg = [None] * NBLOCKS
        invs = [None] * NBLOCKS
        fps = [None] * NBLOCKS
        saccs = [None] * NBLOCKS

        # ---------- block 0: Q'(0) + gate(0) first, Q'(1..3) interleaved ----
        with nc.named_scope("blk0"):
            emit_qproj(0)
            tg[0] = emit_gate(0)

            def boundary0(k):
                if k in (3, 6, 9):
                    emit_qproj(k // 3)

            fps[0], saccs[0] = emit_block(0, boundary0)

        # ---------- blocks 1..3 with previous block's post interleaved ----
        for j in range(1, NBLOCKS):
            p = j - 1

            def boundary(k, p=p, j=j):
                # PE-order interleave: gate(j) early, then S/M1 of block p
                # spaced between logits steps so PE never waits.
                if k == 0:
                    tg[j] = emit_gate(j)
                elif k == 1:
                    invs[p] = s_finalize(p, saccs[p])
                elif k == 2:
                    Fs = [work.tile([128, NBLK], BF16, tag=f"Fs{co}",
                                    name=f"Fs{co}", bufs=2) for co in range(2)]
                    for co in range(2):
                        nc.vector.tensor_copy(Fs[co][:], fps[p][co][:])
                    fps[p] = Fs
                elif k == 3:
                    mp = emit_m1(fps[p], 0)
                    post_co(p, 0, mp, invs[p], tg[p])
                elif k == 4:
                    mp = emit_m1(fps[p], 1)
                    post_co(p, 1, mp, invs[p], tg[p])

            with nc.named_scope(f"blk{j}"):
                fps[j], saccs[j] = emit_block(j, boundary)

        # ---------- tail: block 3 post ----------
        p = NBLOCKS - 1
        with nc.named_scope("tail"):
            invs[p] = s_finalize(p, saccs[p])
            Fs = [work.tile([128, NBLK], BF16, tag=f"Fs{co}", name=f"Fs{co}",
                            bufs=2) for co in range(2)]
            for co in range(2):
                nc.vector.tensor_copy(Fs[co][:], fps[p][co][:])
            for co in range(2):
                mp = emit_m1(Fs, co)
                post_co(p, co, mp, invs[p], tg[p])
    nc.compile()
    return nc


_NC = None


def _get_nc():
    global _NC
    if _NC is None:
        _NC = build()
    return _NC


def kernel(**inputs):
    x1 = np.ascontiguousarray(np.asarray(inputs["x1"], dtype=np.float32)).reshape(B, C, N)
    x2 = np.ascontiguousarray(np.asarray(inputs["x2"], dtype=np.float32)).reshape(B, C, N)
    q_w = np.asarray(inputs["q_w"], np.float64)
    k_w = np.asarray(inputs["k_w"], np.float64)
    v_w = np.asarray(inputs["v_w"], np.float64)
    p_w = np.asarray(inputs["proj_w"], np.float64)
    q_b = np.asarray(inputs["q_b"], np.float64)
    v_b = np.asarray(inputs["v_b"], np.float64)
    p_b = np.asarray(inputs["proj_b"], np.float64)
    gamma = np.asarray(inputs["bn_gamma"], np.float64)
    beta = np.asarray(inputs["bn_beta"], np.float64)
    mean = np.asarray(inputs["bn_mean"], np.float64)
    var = np.asarray(inputs["bn_var"], np.float64)
    gate_w = np.asarray(inputs["gate_w"], np.float64)
    gate_b = np.asarray(inputs["gate_b"], np.float64)

    # folded weights: Q' = (k_w^T q_w) x1 + k_w^T q_b ;  M1 = (proj_w v_w) Z
    wq = np.ascontiguousarray(np.asarray(q_w.T @ k_w, np.float32))
    p2 = np.asarray(v_w.T @ p_w.T, np.float32).astype(ml_dtypes.bfloat16)
    # gate lhsT, replicated along the output-partition dim
    gwf = np.ascontiguousarray(
        np.repeat(gate_w[0, :C].astype(np.float32)[:, None], 128, axis=1))
    gwb = np.ascontiguousarray(
        np.repeat(gate_w[0, C:].astype(np.float32)[:, None], 128,
                  axis=1)).astype(ml_dtypes.bfloat16)
    G = gamma / np.sqrt(var + EPS)
    Bc = beta + (p_b + p_w @ v_b - mean) * G
    qpb = k_w.T @ q_b
    gb2 = np.full(C, float(gate_b[0]) * 0.5)
    vecs = np.ascontiguousarray(
        np.stack([qpb, G * 0.5, Bc * 0.5, gb2], axis=1).astype(np.float32))

    in_maps = []
    for core in range(NCORES):
        b, half = divmod(core, 2)
        hq = slice(half * NH, (half + 1) * NH)
        ho = slice((1 - half) * NH, (2 - half) * NH)
        x1q = np.ascontiguousarray(x1[b][:, hq])
        x2p = np.ascontiguousarray(np.concatenate([x2[b][:, hq], x2[b][:, ho]],
                                                  axis=1))
        x2pb = x2p.astype(ml_dtypes.bfloat16)
        # x2 pretransposed into the fusion lhsT SBUF layout:
        # x2tb[p, mt*C + c] = x2p[c, mt*128 + p]
        x2tb = np.ascontiguousarray(
            x2pb.reshape(C, MT, 128).transpose(2, 1, 0).reshape(128, MT * C))
        in_maps.append({
            "x1r": x1q, "x2rb": np.ascontiguousarray(x2pb), "x2tb": x2tb,
            "wq": wq, "p2": p2, "gwf": gwf, "gwb": gwb, "vecs": vecs,
        })

    nc = _get_nc()
    res = run_bass_kernel_spmd(nc, in_maps, core_ids=list(range(NCORES)))
    out = np.empty((B, C, N), np.float32)
    for core in range(NCORES):
        b, half = divmod(core, 2)
        out[b, :, half * NH:(half + 1) * NH] = res.results[core]["out"]
    return out.reshape(B, C, H, W)


# revision 6
# speedup vs baseline: 1.1841x; 1.0064x over previous
"""CrossAttentionFusion Trainium2 kernel (v3).

Full inputs -> shard (batch x query-half) over 8 NeuronCores -> full output.

Per core (batch b = core//2, query half h = core%2, NH=2048 queries):
  Algebraic folding (host precompute):
    L[m,n] = K^T Q = x2^T (k_w^T q_w) x1 =: x2^T Q'   (K never materialized;
             x2^T k_w^T q_b folds into Q' channel bias)
    F_att   = v_w (x2 A_norm) + v_b  ->  M1 = (proj_w v_w) Z,  Z = x2 E
    gate    = sigmoid(z) = (1 + tanh(z/2)) / 2; the 1/2 folds into the BN
             constants so ACT never leaves the exp/tanh function table.
  Device per 512-query block j (fusion interleaved INTO the same block,
  trailing exp by 2 key-tile-pairs; E is a 4-slot ring, not a full buffer):
    L[m, ns] = x2^T Q'            (bf16 matmuls, keys m on partitions)
    E = exp(L / 16)               (ACT -> bf16; logits O(1), no max needed)
    Z[c, ns] = sum_m x2t[m,c] E[m, ns]   (bf16, accumulated over 32 m-tiles)
    S[ns] = sum_m E[m, ns]        (bf16 pairwise tree on DVE (2x mode) down to
                                   2 partials, then 2 accumulating
                                   ones[128,128] matmuls -> S broadcast to all
                                   partitions; reciprocal on DVE)
    M1 = P2 Z ; r' = relu(M1*(G/2)*(1/S) + Bc/2)  (DVE STT + ACT relu-bias)
    out = x1 + (1+tanh((gz+gb)/2)) * r'           (DVE STT + Pool add)
  with G = gamma*rsqrt(var+eps), Bc = beta + (proj_b + proj_w v_b - mean)*G.
  All matmul inputs ship as bf16 (x1 additionally as fp32, streamed late,
  only for the residual add).  x2 ships twice: channels-major for logits,
  keys-major pretransposed for fusion.  DMA uses 3 DGE rings (SP + ACT +
  Pool), chunks ordered by first use.  The final block's post chain runs in
  256-column chunks pipelined across DVE/ACT/Pool to shrink the tail.
"""
from contextlib import ExitStack

import numpy as np
import ml_dtypes

import concourse.bass as bass
import concourse.mybir as mybir
import concourse.tile as tile
from concourse import bacc
from concourse.bass_utils import run_bass_kernel_spmd

F32 = mybir.dt.float32
F32R = mybir.dt.float32r
BF16 = mybir.dt.bfloat16
AF = mybir.ActivationFunctionType
OP = mybir.AluOpType

B, C, H, W = 4, 256, 64, 64
N = H * W            # 4096
NCORES = 8
NH = N // 2          # 2048 queries per core
NBLK = 512           # query block
NBLOCKS = NH // NBLK
MT = N // 128        # 32 key tiles
MT2 = MT // 2        # 16 exp steps per block
EPS = 1e-5
SCALE = float(C) ** -0.5


def build():
    nc = bacc.Bacc("TRN2", target_bir_lowering=False, debug=False,
                   num_devices=NCORES)
    x1f_d = nc.dram_tensor("x1f", [C, NH], F32R, kind="ExternalInput")
    x1b_d = nc.dram_tensor("x1b", [C, NH], BF16, kind="ExternalInput")
    x2rb_d = nc.dram_tensor("x2rb", [C, N], BF16, kind="ExternalInput")
    x2tb_d = nc.dram_tensor("x2tb", [128, MT * C], BF16, kind="ExternalInput")
    wq_d = nc.dram_tensor("wq", [C, C], BF16, kind="ExternalInput")
    p2_d = nc.dram_tensor("p2", [C, C], BF16, kind="ExternalInput")
    gw_d = nc.dram_tensor("gw", [C, 256], BF16, kind="ExternalInput")
    vec_d = nc.dram_tensor("vecs", [C, 4], F32, kind="ExternalInput")
    out_d = nc.dram_tensor("out", [C, NH], F32, kind="ExternalOutput")

    with tile.TileContext(nc) as tc, ExitStack() as ctx:
        pers = ctx.enter_context(tc.tile_pool(name="pers", bufs=1))
        work = ctx.enter_context(tc.tile_pool(name="work", bufs=2))
        psum = ctx.enter_context(tc.tile_pool(name="psum", bufs=1, space="PSUM"))

        # ---- persistent tiles ----
        wq = [pers.tile([128, C], BF16, tag=f"wq{ci}", name=f"wq{ci}") for ci in range(2)]
        p2 = [pers.tile([128, C], BF16, tag=f"p2{ci}", name=f"p2{ci}") for ci in range(2)]
        gw = [pers.tile([128, 256], BF16, tag=f"gw{ci}", name=f"gw{ci}") for ci in range(2)]
        vec = [pers.tile([128, 4], F32, tag=f"vec{ci}", name=f"vec{ci}") for ci in range(2)]
        x1f = [pers.tile([128, NH], F32R, tag=f"x1f{ci}", name=f"x1f{ci}") for ci in range(2)]
        x1b = [pers.tile([128, NH], BF16, tag=f"x1b{ci}", name=f"x1b{ci}") for ci in range(2)]
        x2rb = [pers.tile([128, N], BF16, tag=f"x2rb{ci}", name=f"x2rb{ci}") for ci in range(2)]
        x2tb = pers.tile([128, MT * C], BF16, tag="x2tb", name="x2tb")
        Qt = [pers.tile([128, NH], BF16, tag=f"Qt{co}", name=f"Qt{co}") for co in range(2)]
        ones_f = pers.tile([128, 128], F32, tag="ones_f", name="ones_f")
        ones_b = pers.tile([128, 128], BF16, tag="ones_b", name="ones_b")

        XCH = 512
        NX1 = NH // XCH   # 4 chunks per ci
        NX2 = N // XCH    # 8 chunks per ci
        c0, c1 = slice(0, 128), slice(128, 256)
        cs2 = [c0, c1]

        # ---------- pre: constants + input streaming (3 DGE rings) ----------
        # per-ring order == need order; first-matmul deps are the first,
        # smallest transfers on each ring.
        with nc.named_scope("pre"):
            nc.vector.memset(ones_f[:], 1.0)
            nc.vector.tensor_copy(ones_b[:], ones_f[:])
            s0 = slice(0, XCH)
            # ring SP: wq -> x2rb ci0 stream (+ x1b ci0 rest, x1f ci0 woven)
            nc.sync.dma_start(wq[0][:], wq_d[c0, :])
            nc.sync.dma_start(wq[1][:], wq_d[c1, :])
            # ring Pool: x1b ch0 -> vec -> gw -> x2rb ci1 stream (+ x1f ci1)
            nc.gpsimd.dma_start(x1b[0][:, s0], x1b_d[c0, s0])
            nc.gpsimd.dma_start(x1b[1][:, s0], x1b_d[c1, s0])
            nc.gpsimd.dma_start(vec[0][:], vec_d[c0, :])
            nc.gpsimd.dma_start(vec[1][:], vec_d[c1, :])
            nc.sync.dma_start(x2rb[0][:, s0], x2rb_d[c0, s0])
            nc.gpsimd.dma_start(gw[0][:], gw_d[c0, :])
            nc.gpsimd.dma_start(gw[1][:], gw_d[c1, :])
            nc.gpsimd.dma_start(x2rb[1][:, s0], x2rb_d[c1, s0])
            # ring ACT: fusion-side lhsT stream
            nc.scalar.dma_start(x2tb[:, 0:4 * C], x2tb_d[:, 0:4 * C])
            for ch in range(1, NX2):
                chs = slice(ch * XCH, (ch + 1) * XCH)
                if ch < NX1:
                    nc.sync.dma_start(x1b[0][:, chs], x1b_d[c0, chs])
                    nc.gpsimd.dma_start(x1b[1][:, chs], x1b_d[c1, chs])
                nc.sync.dma_start(x2rb[0][:, chs], x2rb_d[c0, chs])
                nc.gpsimd.dma_start(x2rb[1][:, chs], x2rb_d[c1, chs])
                ts = slice(ch * 4 * C, (ch + 1) * 4 * C)
                nc.scalar.dma_start(x2tb[:, ts], x2tb_d[:, ts])
                if ch % 2 == 1:  # x1f block (ch-1)//2, both ci, woven in
                    blk = (ch - 1) // 2
                    bs = slice(blk * NBLK, (blk + 1) * NBLK)
                    nc.sync.dma_start(x1f[0][:, bs], x1f_d[c0, bs])
                    nc.gpsimd.dma_start(x1f[1][:, bs], x1f_d[c1, bs])
            nc.scalar.dma_start(p2[0][:], p2_d[c0, :])
            nc.scalar.dma_start(p2[1][:], p2_d[c1, :])
            for blk in range(NX2 // 2 - 1, NBLOCKS):
                bs = slice(blk * NBLK, (blk + 1) * NBLK)
                nc.sync.dma_start(x1f[0][:, bs], x1f_d[c0, bs])
                nc.gpsimd.dma_start(x1f[1][:, bs], x1f_d[c1, bs])

        def emit_qproj(nch):
            ns = slice(nch * NBLK, (nch + 1) * NBLK)
            for co in range(2):
                qp = psum.tile([128, NBLK], F32, tag="acc", name="acc", bufs=2)
                for ci in range(2):
                    nc.tensor.matmul(
                        qp[:], wq[ci][:, co * 128:(co + 1) * 128],
                        x1b[ci][:, ns], start=(ci == 0), stop=(ci == 1))
                nc.scalar.activation(Qt[co][:, ns], qp[:], AF.Identity,
                                     bias=vec[co][:, 0:1])

        def emit_gate(j):
            """Gate logits for block j, partition-broadcast via replicated
            gate-weight lhsT; tanh((z+gb)/2) -> tg [128,NBLK] fp32."""
            ns = slice(j * NBLK, (j + 1) * NBLK)
            gp = psum.tile([128, NBLK], F32, tag="acc", name="gp", bufs=2)
            for ci in range(2):
                nc.tensor.matmul(gp[:], gw[ci][:, 0:128], x1b[ci][:, ns],
                                 start=(ci == 0), stop=False)
            for ci in range(2):
                nc.tensor.matmul(gp[:], gw[ci][:, 128:256], x2rb[ci][:, ns],
                                 start=False, stop=(ci == 1))
            tg = work.tile([128, NBLK], F32, tag="tg", name="tg", bufs=2)
            nc.scalar.activation(tg[:], gp[:], AF.Tanh, scale=0.5,
                                 bias=vec[0][:, 3:4])
            return tg

        def s_finalize(j, sacc2):
            """S (sum over keys) broadcast to all partitions, then 1/S."""
            with nc.named_scope(f"sfin{j}"):
                sb = psum.tile([128, NBLK], F32, tag="acc", name="sb", bufs=2)
                nc.tensor.matmul(sb[:], ones_b[:], sacc2[0][:], start=True,
                                 stop=False)
                nc.tensor.matmul(sb[:], ones_b[:], sacc2[1][:], start=False,
                                 stop=True)
                invs = work.tile([128, NBLK], F32, tag="invs", name="invs",
                                 bufs=2)
                nc.vector.reciprocal_approx_fast(invs[:], sb[:])
            return invs

        def emit_m1(Fs, co):
            mp = psum.tile([128, NBLK], F32, tag="acc", name="acc", bufs=2)
            for ci in range(2):
                nc.tensor.matmul(mp[:], p2[ci][:, co * 128:(co + 1) * 128],
                                 Fs[ci][:], start=(ci == 0), stop=(ci == 1))
            return mp

        def post_co(j, co, mp, invs, tg):
            """Normalize + BN + relu + gate + residual + store for one co."""
            ns = slice(j * NBLK, (j + 1) * NBLK)
            cs = cs2[co]
            with nc.named_scope(f"post{j}_{co}"):
                t1 = work.tile([128, NBLK], F32, tag=f"t1{co}", name="t1")
                nc.vector.scalar_tensor_tensor(
                    t1[:], mp[:], vec[co][:, 1:2], invs[:],
                    op0=OP.mult, op1=OP.mult)
                r = work.tile([128, NBLK], F32, tag=f"r{co}", name="r")
                nc.scalar.activation(r[:], t1[:], AF.Relu,
                                     bias=vec[co][:, 2:3])
                rg = work.tile([128, NBLK], F32, tag=f"rg{co}", name="rg")
                nc.vector.scalar_tensor_tensor(rg[:], tg[:], 1.0, r[:],
                                               op0=OP.add, op1=OP.mult)
                ot = work.tile([128, NBLK], F32, tag=f"ot{co}", name="ot")
                nc.gpsimd.tensor_add(ot[:], rg[:],
                                     x1f[co][:, ns].bitcast(F32))
                nc.sync.dma_start(out_d[cs, ns], ot[:])

        def post_tail(j, co, mp, invs, tg):
            """Tail post: 256-col chunks pipelined across DVE/ACT/Pool."""
            HB = NBLK // 2
            for h in range(2):
                hs = slice(h * HB, (h + 1) * HB)
                ns = slice(j * NBLK + h * HB, j * NBLK + (h + 1) * HB)
                with nc.named_scope(f"post{j}_{co}"):
                    t1 = work.tile([128, HB], F32, tag=f"tt{co}{h}", name="t1")
                    nc.vector.scalar_tensor_tensor(
                        t1[:], mp[:, hs], vec[co][:, 1:2], invs[:, hs],
                        op0=OP.mult, op1=OP.mult)
                    r = work.tile([128, HB], F32, tag=f"tr{co}{h}", name="r")
                    nc.scalar.activation(r[:], t1[:], AF.Relu,
                                         bias=vec[co][:, 2:3])
                    rg = work.tile([128, HB], F32, tag=f"tg{co}{h}", name="rg")
                    nc.vector.scalar_tensor_tensor(rg[:], tg[:, hs], 1.0,
                                                   r[:], op0=OP.add,
                                                   op1=OP.mult)
                    ot = work.tile([128, HB], F32, tag=f"to{co}{h}", name="ot")
                    if (co + h) % 2 == 0:
                        nc.gpsimd.tensor_add(ot[:], rg[:],
                                             x1f[co][:, ns].bitcast(F32))
                    else:
                        nc.vector.tensor_add(ot[:], rg[:],
                                             x1f[co][:, ns].bitcast(F32))
                    nc.sync.dma_start(out_d[cs2[co], ns], ot[:])

        def emit_block(j, boundary):
            """Logits+exp+fusion for block j; fusion trails exp by 2 steps.
            boundary(k) emits interleaved PE work after logits step k."""
            ns = slice(j * NBLK, (j + 1) * NBLK)
            slots = [None] * 6
            sacc2 = []

            def feed(t, lvl):
                if lvl == 5:
                    sacc2.append(t)
                    return
                if slots[lvl] is None:
                    slots[lvl] = t
                    return
                prev = slots[lvl]
                slots[lvl] = None
                nt = work.tile([128, NBLK], BF16, tag=f"tree{lvl}",
                               name=f"tree{lvl}", bufs=2)
                nc.vector.tensor_add(nt[:], prev[:], t[:])
                feed(nt, lvl + 1)

            fp = [psum.tile([128, NBLK], F32, tag=f"F{co}", name=f"F{co}",
                            bufs=1) for co in range(2)]
            Ets = [None] * MT2

            def fusion_step(mt2):
                Et = Ets[mt2]
                for sub in range(2):
                    mt = 2 * mt2 + sub
                    es = slice(sub * NBLK, (sub + 1) * NBLK)
                    for co in range(2):
                        nc.tensor.matmul(
                            fp[co][:],
                            x2tb[:, mt * C + co * 128: mt * C + (co + 1) * 128],
                            Et[:, es], start=(mt == 0), stop=(mt == MT - 1))

            for mt2 in range(MT2):
                lp = psum.tile([128, 2 * NBLK], F32, tag="L", name="L", bufs=2)
                for sub in range(2):
                    mt = 2 * mt2 + sub
                    msl = slice(mt * 128, (mt + 1) * 128)
                    for ci in range(2):
                        nc.tensor.matmul(
                            lp[:, sub * NBLK:(sub + 1) * NBLK],
                            x2rb[ci][:, msl], Qt[ci][:, ns],
                            start=(ci == 0), stop=(ci == 1))
                if boundary is not None:
                    boundary(mt2)
                Et = work.tile([128, 2 * NBLK], BF16, tag="E", name="E",
                               bufs=4)
                nc.scalar.activation(Et[:], lp[:], AF.Exp, scale=SCALE)
                Ets[mt2] = Et
                if mt2 >= 2:
                    fusion_step(mt2 - 2)
                # softmax-sum tree (DVE, bf16 2x): pair within Et, then fold
                p1 = work.tile([128, NBLK], BF16, tag="tree1", name="tree1",
                               bufs=2)
                nc.vector.tensor_add(p1[:], Et[:, 0:NBLK], Et[:, NBLK:2 * NBLK])
                feed(p1, 2)
            fusion_step(MT2 - 2)
            fusion_step(MT2 - 1)
            assert len(sacc2) == 2
            return fp, sacc2

        tg = [None] * NBLOCKS
        invs = [None] * NBLOCKS
        fps = [None] * NBLOCKS
        saccs = [None] * NBLOCKS

        # ---------- block 0: Q'(0) + gate(0) first, Q'(1..3) interleaved ----
        with nc.named_scope("blk0"):
            emit_qproj(0)
            tg[0] = emit_gate(0)

            def boundary0(k):
                if k in (3, 6, 9):
                    emit_qproj(k // 3)

            fps[0], saccs[0] = emit_block(0, boundary0)

        # ---------- blocks 1..3 with previous block's post interleaved ----
        for j in range(1, NBLOCKS):
            p = j - 1

            def boundary(k, p=p, j=j):
                # PE-order interleave: gate(j) early, then S/M1 of block p
                # spaced between logits steps so PE never waits.
                if k == 0:
                    tg[j] = emit_gate(j)
                elif k == 1:
                    invs[p] = s_finalize(p, saccs[p])
                elif k == 2:
                    Fs = [work.tile([128, NBLK], BF16, tag=f"Fs{co}",
                                    name=f"Fs{co}", bufs=2) for co in range(2)]
                    for co in range(2):
                        nc.vector.tensor_copy(Fs[co][:], fps[p][co][:])
                    fps[p] = Fs
                elif k == 3:
                    mp = emit_m1(fps[p], 0)
                    post_co(p, 0, mp, invs[p], tg[p])
                elif k == 4:
                    mp = emit_m1(fps[p], 1)
                    post_co(p, 1, mp, invs[p], tg[p])

            with nc.named_scope(f"blk{j}"):
                fps[j], saccs[j] = emit_block(j, boundary)

        # ---------- tail: block 3 post, chunked + ACT evictions ----------
        p = NBLOCKS - 1
        with nc.named_scope("tail"):
            Fs = [work.tile([128, NBLK], BF16, tag=f"Fs{co}", name=f"Fs{co}",
                            bufs=2) for co in range(2)]
            for co in range(2):
                nc.scalar.activation(Fs[co][:], fps[p][co][:], AF.Copy)
            invs[p] = s_finalize(p, saccs[p])
            mps = [emit_m1(Fs, co) for co in range(2)]
            for co in range(2):
                post_tail(p, co, mps[co], invs[p], tg[p])
    nc.compile()
    return nc


_NC = None


def _get_nc():
    global _NC
    if _NC is None:
        _NC = build()
    return _NC


def kernel(**inputs):
    x1 = np.ascontiguousarray(np.asarray(inputs["x1"], dtype=np.float32)).reshape(B, C, N)
    x2 = np.ascontiguousarray(np.asarray(inputs["x2"], dtype=np.float32)).reshape(B, C, N)
    q_w = np.asarray(inputs["q_w"], np.float64)
    k_w = np.asarray(inputs["k_w"], np.float64)
    v_w = np.asarray(inputs["v_w"], np.float64)
    p_w = np.asarray(inputs["proj_w"], np.float64)
    q_b = np.asarray(inputs["q_b"], np.float64)
    v_b = np.asarray(inputs["v_b"], np.float64)
    p_b = np.asarray(inputs["proj_b"], np.float64)
    gamma = np.asarray(inputs["bn_gamma"], np.float64)
    beta = np.asarray(inputs["bn_beta"], np.float64)
    mean = np.asarray(inputs["bn_mean"], np.float64)
    var = np.asarray(inputs["bn_var"], np.float64)
    gate_w = np.asarray(inputs["gate_w"], np.float64)
    gate_b = np.asarray(inputs["gate_b"], np.float64)

    # folded weights: Q' = (k_w^T q_w) x1 + k_w^T q_b ;  M1 = (proj_w v_w) Z
    wq = np.asarray(q_w.T @ k_w, np.float32).astype(ml_dtypes.bfloat16)
    p2 = np.asarray(v_w.T @ p_w.T, np.float32).astype(ml_dtypes.bfloat16)
    # gate lhsT, replicated along the output-partition dim: [x1 part | x2 part]
    gwrep = np.concatenate([
        np.repeat(gate_w[0, :C].astype(np.float32)[:, None], 128, axis=1),
        np.repeat(gate_w[0, C:].astype(np.float32)[:, None], 128, axis=1),
    ], axis=1).astype(ml_dtypes.bfloat16)
    G = gamma / np.sqrt(var + EPS)
    Bc = beta + (p_b + p_w @ v_b - mean) * G
    qpb = k_w.T @ q_b
    gb2 = np.full(C, float(gate_b[0]) * 0.5)
    vecs = np.ascontiguousarray(
        np.stack([qpb, G * 0.5, Bc * 0.5, gb2], axis=1).astype(np.float32))

    in_maps = []
    for core in range(NCORES):
        b, half = divmod(core, 2)
        hq = slice(half * NH, (half + 1) * NH)
        ho = slice((1 - half) * NH, (2 - half) * NH)
        x1q = np.ascontiguousarray(x1[b][:, hq])
        x2p = np.ascontiguousarray(np.concatenate([x2[b][:, hq], x2[b][:, ho]],
                                                  axis=1))
        x2pb = x2p.astype(ml_dtypes.bfloat16)
        # x2 pretransposed into the fusion lhsT SBUF layout:
        # x2tb[p, mt*C + c] = x2p[c, mt*128 + p]
        x2tb = np.ascontiguousarray(
            x2pb.reshape(C, MT, 128).transpose(2, 1, 0).reshape(128, MT * C))
        in_maps.append({
            "x1f": x1q, "x1b": x1q.astype(ml_dtypes.bfloat16),
            "x2rb": np.ascontiguousarray(x2pb), "x2tb": x2tb,
            "wq": wq, "p2": p2, "gw": gwrep, "vecs": vecs,
        })

    nc = _get_nc()
    res = run_bass_kernel_spmd(nc, in_maps, core_ids=list(range(NCORES)))
    out = np.empty((B, C, N), np.float32)
    for core in range(NCORES):
        b, half = divmod(core, 2)
        out[b, :, half * NH:(half + 1) * NH] = res.results[core]["out"]
    return out.reshape(B, C, H, W)
